# revision 1
# baseline (speedup 1.0000x reference)
"""Single-head causal attention (B=256, T=256, C=1024, D=64) on 8 TRN2 NeuronCores.

Strategy: data-parallel over batch (32 batches/core). Host pre-transposes x to
x^T [B, C, T] bf16 so the contraction dim C lands on SBUF partitions with fully
contiguous DMA, eliminating every on-device transpose:

  per batch b (all layouts partition-major):
    qk^T [128,T]  = [Wq|Wk]^T @ x_b^T        (8 accumulating matmuls, K=128)
    sc^T [S,T]    = k @ q^T                   (2 matmuls, K=64; the fully-masked
                                               (s>=128, t<128) quadrant is skipped)
    e^T           = exp(sc^T / 32)            (one ScalarE op over [128, 384])
    e^T quadrants causal-zeroed via GpSimd affine_select (only the 2 diagonal
                                               [128,128] quadrants need masking)
    v    [S,D]    = (x_b^T)^T @ Wv            (16 matmuls, xT-stationary)
    o'   [T,2,D+1]= e^T.T @ [v | 1]           (numerator and softmax denominator
                                               fused into one accumulation)
    out  [T,D]    = o' * (1/den)              (VectorE reciprocal + broadcast mult)

The final stage of batch b is emitted during batch b+1 (software pipelining) so
the in-order PE stream never stalls on the exp/mask chain. Softmax needs no
max-subtraction: scores/32 ~ N(0, 0.25^2), |max| < ~1.6, so exp never overflows.
"""

import numpy as np
import ml_dtypes

import concourse.bacc as bacc
import concourse.mybir as mybir
import concourse.tile as tile
from concourse.bass_utils import run_bass_kernel_spmd

B, T, C, D = 256, 256, 1024, 64
NCORES = 8
BPC = B // NCORES  # batches per core
CCH = C // 128  # contraction chunks
SCALE = float(C) ** -0.5

BF16 = mybir.dt.bfloat16
F32 = mybir.dt.float32

TRACE = False
LAST_RESULT = None


def _build(
    xp_bufs=6,
    ep_bufs=4,
    vp_bufs=4,
    qk_ps_bufs=2,
    sc_ps_bufs=3,
    v_ps_bufs=2,
    o_ps_bufs=1,
    alt_rings=False,
    split_xt=True,
    xt_ways=2,
    split_out=False,
    early_final=False,
):
    nc = bacc.Bacc(
        "TRN2", target_bir_lowering=False, debug=False, num_devices=NCORES
    )
    xt = nc.dram_tensor("xt", [BPC, C, T], BF16, kind="ExternalInput")
    wqk = nc.dram_tensor("wqk", [C, 128], BF16, kind="ExternalInput")
    wv = nc.dram_tensor("wv", [C, D], BF16, kind="ExternalInput")
    out = nc.dram_tensor("out", [BPC, T, D], F32, kind="ExternalOutput")

    with tile.TileContext(nc) as tc:
        with (
            tc.tile_pool(name="singles", bufs=1) as singles,
            tc.tile_pool(name="xp", bufs=xp_bufs) as xp,
            tc.tile_pool(name="sbp", bufs=3) as sbp,
            tc.tile_pool(name="ep", bufs=ep_bufs) as ep,
            tc.tile_pool(name="vp", bufs=vp_bufs) as vp,
            tc.tile_pool(name="outp", bufs=4) as outp,
            tc.tile_pool(name="qk_ps", bufs=qk_ps_bufs, space="PSUM") as qk_psp,
            tc.tile_pool(name="sc_ps", bufs=sc_ps_bufs, space="PSUM") as sc_psp,
            tc.tile_pool(name="v_ps", bufs=v_ps_bufs, space="PSUM") as v_psp,
            tc.tile_pool(name="o_ps", bufs=o_ps_bufs, space="PSUM") as o_psp,
        ):
            wqk_sb = singles.tile([128, CCH, 128], BF16)
            nc.sync.dma_start(wqk_sb, wqk[:].rearrange("(c p) m -> p c m", p=128))
            wv_sb = singles.tile([128, CCH, D], BF16)
            nc.sync.dma_start(wv_sb, wv[:].rearrange("(c p) m -> p c m", p=128))

            def final_stage(b, expT, v_sb):
                """Final matmuls + softmax normalization + out-DMA for batch b.

                Emitted one batch late so the PE work here never waits on the
                (ACT exp / Pool mask) chain of the same batch.
                """
                # o2[:, tt, :] = [num | den] for t-tile tt; one PSUM bank.
                o2 = o_psp.tile([128, 2, D + 1], F32, tag="o_ps")
                nc.tensor.matmul(
                    o2[:, 0],
                    lhsT=expT[:, 0:128],
                    rhs=v_sb[:, 0],
                    start=True,
                    stop=True,
                )
                nc.tensor.matmul(
                    o2[:, 1],
                    lhsT=expT[:, 128:256],
                    rhs=v_sb[:, 0],
                    start=True,
                    stop=False,
                )
                nc.tensor.matmul(
                    o2[:, 1],
                    lhsT=expT[:, 256:384],
                    rhs=v_sb[:, 1],
                    start=False,
                    stop=True,
                )
                out_v = out[b].rearrange("(tt p) d -> p tt d", p=128)
                if split_out:
                    # normalize + store per t-tile so the first out-DMA is
                    # ready as soon as tt=0's matmul lands
                    for tt in range(2):
                        recip = outp.tile([128, 1], F32, tag="recip")
                        nc.vector.reciprocal(recip, o2[:, tt, D : D + 1])
                        o_sb = outp.tile([128, D], F32, tag="o_sb")
                        nc.vector.tensor_scalar_mul(o_sb, o2[:, tt, 0:D], recip)
                        nc.scalar.dma_start(out_v[:, tt], o_sb)
                else:
                    recip = outp.tile([128, 2], F32, tag="recip")
                    nc.vector.reciprocal(recip, o2[:, :, D])
                    o_sb = outp.tile([128, 2, D], F32, tag="o_sb")
                    nc.vector.tensor_tensor(
                        o_sb,
                        o2[:, :, 0:D],
                        recip[:, :, None].to_broadcast((128, 2, D)),
                        mybir.AluOpType.mult,
                    )
                    nc.scalar.dma_start(out_v, o_sb)

            prev = None
            for b in range(BPC):
                xt_t = xp.tile([128, CCH, T], BF16, tag="xt")
                dma_eng = nc.scalar if (alt_rings and b % 2) else nc.sync
                xt_src = xt[b].rearrange("(c p) t -> p c t", p=128)
                if split_xt:
                    w = CCH // xt_ways
                    for i in range(xt_ways):
                        dma_eng.dma_start(
                            xt_t[:, i * w : (i + 1) * w],
                            xt_src[:, i * w : (i + 1) * w],
                        )
                else:
                    dma_eng.dma_start(xt_t, xt_src)

                qk_ps = qk_psp.tile([128, T], F32, tag="qk")
                for c in range(CCH):
                    nc.tensor.matmul(
                        qk_ps,
                        lhsT=wqk_sb[:, c],
                        rhs=xt_t[:, c],
                        start=(c == 0),
                        stop=(c == CCH - 1),
                    )

                if early_final and prev is not None:
                    # emit last batch's finale right after this batch's qk
                    # matmuls: its out-DMA issue isn't queued behind this
                    # batch's exp on ACT, and its PE matmuls widen the window
                    # for the DVE q/k copies before the scores matmuls.
                    final_stage(*prev)

                # v projection (independent of scores chain; keeps PE busy
                # while DVE copies q/k out of PSUM)
                v_sb = vp.tile([128, 2, D + 1], BF16, tag="v")
                for st in range(2):
                    v_ps = v_psp.tile([128, D], F32, tag="v_ps")
                    for c in range(CCH):
                        nc.tensor.matmul(
                            v_ps,
                            lhsT=xt_t[:, c, st * 128 : (st + 1) * 128],
                            rhs=wv_sb[:, c],
                            start=(c == 0),
                            stop=(c == CCH - 1),
                        )
                    nc.scalar.copy(v_sb[:, st, 0:D], v_ps)
                    nc.gpsimd.memset(v_sb[:, st, D : D + 1], 1.0)

                q_sb = sbp.tile([64, T], BF16, tag="q_sb")
                nc.vector.tensor_copy(q_sb, qk_ps[0:64, :])
                k_sb = sbp.tile([64, T], BF16, tag="k_sb")
                nc.vector.tensor_copy(k_sb, qk_ps[64:128, :])

                # scores^T, packed in one PSUM bank [128, 384]:
                # cols 0:256   = st=0 (all t)
                # cols 256:384 = st=1, t in [128, 256)
                # (the (st=1, t<128) quadrant is fully causal-masked, skipped)
                sc_ps = sc_psp.tile([128, 3 * 128], F32, tag="sc")
                nc.tensor.matmul(
                    sc_ps[:, 0:T],
                    lhsT=k_sb[:, 0:128],
                    rhs=q_sb[:],
                    start=True,
                    stop=True,
                )
                nc.tensor.matmul(
                    sc_ps[:, T : T + 128],
                    lhsT=k_sb[:, 128:T],
                    rhs=q_sb[:, 128:T],
                    start=True,
                    stop=True,
                )
                expT = ep.tile([128, 3 * 128], BF16, tag="expT")
                nc.scalar.activation(
                    expT,
                    sc_ps,
                    func=mybir.ActivationFunctionType.Exp,
                    scale=SCALE,
                )
                # triangular causal mask on the two diagonal quadrants
                # (the (st=0, t>=128) quadrant is fully unmasked)
                for quad in (0, 256):
                    nc.gpsimd.affine_select(
                        out=expT[:, quad : quad + 128],
                        in_=expT[:, quad : quad + 128],
                        compare_op=mybir.AluOpType.is_ge,
                        fill=0.0,
                        base=0,
                        pattern=[[1, 128]],
                        channel_multiplier=-1,
                    )

                if not early_final and prev is not None:
                    final_stage(*prev)
                prev = (b, expT, v_sb)
            final_stage(*prev)
    nc.compile()
    return nc


def kernel(x: np.ndarray, Wq: np.ndarray, Wk: np.ndarray, Wv: np.ndarray) -> np.ndarray:
    global LAST_RESULT
    x = np.asarray(x, dtype=np.float32)
    Wq = np.asarray(Wq, dtype=np.float32)
    Wk = np.asarray(Wk, dtype=np.float32)
    Wv = np.asarray(Wv, dtype=np.float32)
    xt = np.ascontiguousarray(np.transpose(x, (0, 2, 1))).astype(ml_dtypes.bfloat16)
    wqk = np.concatenate([Wq, Wk], axis=1).astype(ml_dtypes.bfloat16)
    wv = np.ascontiguousarray(Wv).astype(ml_dtypes.bfloat16)

    nc = _build()
    in_maps = [
        {"xt": xt[i * BPC : (i + 1) * BPC], "wqk": wqk, "wv": wv}
        for i in range(NCORES)
    ]
    res = run_bass_kernel_spmd(
        nc, in_maps, core_ids=list(range(NCORES)), trace=TRACE
    )
    LAST_RESULT = res
    out = np.concatenate([r["out"] for r in res.results], axis=0)
    return out


if __name__ == "__main__":
    x = np.random.randn(B, T, C).astype(np.float32)
    Wq = np.random.randn(C, D).astype(np.float32) * (C**-0.5)
    Wk = np.random.randn(C, D).astype(np.float32) * (C**-0.5)
    Wv = np.random.randn(C, D).astype(np.float32) * (C**-0.5)
    o = kernel(x, Wq, Wk, Wv)
    print(o.shape, o.dtype)

